# revision 1
# baseline (speedup 1.0000x reference)
"""CondConv2d (MoE routed conv) Trainium2 Bass kernel.

Strategy
--------
Data-parallel over batch B=32 across 8 NeuronCores (4 samples/core); the
expert bank + routing params are replicated.  Per core and sample:

  1. x[b] is DMA'd into SBUF as two zero-padded [128, 58*58] tiles
     (C=256 split across 2 partition chunks, H/W padded by 1).
  2. pooled = sum_hw(x)  (DVE free-dim reduce; pad zeros don't matter).
     Routing MLP (relu + softmax over E=4) runs on PE/ACT/DVE; the 4
     routing scalars are partition-broadcast via GPSIMD.
  3. Expert mixing: combined[c,(ij,o)] = sum_e r_e * experts[c,e,(ij,o)]
     as a fused scalar_tensor_tensor chain on DVE.  experts are
     host-relayout'ed to [C, E, 3*3, O] so the SBUF slabs DMA contiguously
     and mixed tiles are directly in matmul-lhsT orientation.
  4. Conv = 9 shifted 1x1 convs accumulated in PSUM: for each output
     chunk [128 o, 448] (8 rows x 56 cols), accumulate 2 (c-chunk) x 9
     (shift) matmuls, N=448, operands bitcast to float32r (full PE rate
     at fp32 storage).
  5. PSUM -> SBUF copies on ACT, then HWDGE DMA to HBM.
"""

import numpy as np
from contextlib import ExitStack

import concourse.bass as bass
import concourse.bacc as bacc
import concourse.mybir as mybir
import concourse.tile as tile
from concourse.bass_utils import run_bass_kernel_spmd

F32 = mybir.dt.float32
F32R = mybir.dt.float32r
AF = mybir.ActivationFunctionType
ALU = mybir.AluOpType
AX = mybir.AxisListType

# Problem shapes (hardcoded per contract).
B, C, H, W = 32, 256, 56, 56
E, O, K = 4, 256, 3
HID = 64
NCORES = 8
BL = B // NCORES          # samples per core
CCH = C // 128            # c partition chunks
OCH = O // 128            # o partition chunks
HP, WP = H + 2, W + 2     # padded
RB_ROWS = 8               # output rows per matmul
NRB = H // RB_ROWS        # 7 row blocks
NBLK = RB_ROWS * W        # 448 = matmul free size
KK = K * K

_CACHE = {}


def _build_program(use_f32r=True, reps=1, loop_n=None):
    nc = bacc.Bacc("TRN2", target_bir_lowering=False, debug=False)

    x_d = nc.dram_tensor("x", [BL, C, HP, WP], F32, kind="ExternalInput").ap()
    ex_d = nc.dram_tensor("experts_t", [C, E, KK, O], F32, kind="ExternalInput").ap()
    # packed routing params: [:,0:64]=rw1t cc0, [:,64:128]=rw1t cc1,
    # [0:64,128]=rb1, [0:64,129:133]=rw2t, [0:1,133:137]=rb2
    RP = 2 * HID + 1 + E + E
    rp_d = nc.dram_tensor("rparams", [128, RP], F32, kind="ExternalInput").ap()
    out_d = nc.dram_tensor("out", [BL, O, H, W], F32, kind="ExternalOutput").ap()

    mmdt = F32R if use_f32r else F32

    with tile.TileContext(nc) as tc, ExitStack() as ctx:
        const_pool = ctx.enter_context(tc.tile_pool(name="const", bufs=1))
        xpad_pool = ctx.enter_context(tc.tile_pool(name="xpad", bufs=2 * CCH))
        xstg_pool = ctx.enter_context(tc.tile_pool(name="xstg", bufs=2))
        comb_pool = ctx.enter_context(tc.tile_pool(name="comb", bufs=2 * CCH))
        scr_pool = ctx.enter_context(tc.tile_pool(name="scr", bufs=1))
        ostg_pool = ctx.enter_context(tc.tile_pool(name="ostg", bufs=4))
        small_pool = ctx.enter_context(tc.tile_pool(name="small", bufs=2))
        cpsum_pool = ctx.enter_context(tc.tile_pool(name="cpsum", bufs=NRB, space="PSUM"))
        mpsum_pool = ctx.enter_context(tc.tile_pool(name="mpsum", bufs=1, space="PSUM"))

        # ---- constants / parameters (preload once, single DMA) ----
        rp_t = const_pool.tile([128, RP], F32, name="rp")
        nc.sync.dma_start(rp_t[:], rp_d[:])
        rw1t_t = [rp_t[:, 0:HID], rp_t[:, HID:2 * HID]]
        rb1_t = rp_t[0:HID, 2 * HID:2 * HID + 1]
        rw2t_t = rp_t[0:HID, 2 * HID + 1:2 * HID + 1 + E]
        rb2_t = rp_t[0:1, 2 * HID + 1 + E:2 * HID + 1 + 2 * E]
        ones_t = const_pool.tile([1, 128], F32, name="ones")
        nc.vector.memset(ones_t[:], 1.0)

        NG = 3                    # ij-groups (of 3 kernel taps each)
        GSZ = KK * O // NG        # 768
        slabs = []   # [cc][e][g] -> [128, GSZ] tile

        def emit_slab_loads():
            # g-major so mixing group 0 (all experts) lands first
            slabs.clear()
            for cc in range(CCH):
                slabs.append([[None] * NG for _ in range(E)])
            exv = ex_d.rearrange("c e k o -> c e (k o)")
            for cc in range(CCH):
                for g in range(NG):
                    for e in range(E):
                        t = const_pool.tile([128, GSZ], F32,
                                            name=f"slab{cc}e{e}g{g}")
                        nc.sync.dma_start(
                            t[:], exv[cc * 128:(cc + 1) * 128, e,
                                      g * GSZ:(g + 1) * GSZ])
                        slabs[cc][e][g] = t

        # per-sample state
        xv = {}       # (b, cc) -> padded x tile viewed [128, HP, WP]
        comb = {}     # (b, cc) -> combined weights [128, KK*O]
        pooled_t = {}  # (b, cc) -> [128, 1] sum over h*w

        FH = HP * WP // 2  # split each chunk's load in halves

        def emit_loads(b):
            for cc in range(CCH):
                xc = x_d[b, cc * 128:(cc + 1) * 128].rearrange(
                    "p h w -> p (h w)")
                ps = []
                if use_f32r:
                    # contiguous HWDGE DMAs at f32 into staging, then DVE
                    # rounding passes (f32 -> f32r) that also produce the
                    # h*w pooled sums via accum_out
                    stg = xstg_pool.tile([128, HP * WP], F32, tag="xstg",
                                         name=f"xs{b}_{cc}")
                    t = xpad_pool.tile([128, HP * WP], F32R, tag="xpad",
                                       name=f"xp{b}_{cc}")
                    for h in range(2):
                        sl = slice(h * FH, (h + 1) * FH)
                        nc.sync.dma_start(stg[:, sl], xc[:, sl])
                        p = small_pool.tile([128, 1], F32, tag="pooled",
                                            bufs=8, name=f"pool{b}_{cc}_{h}")
                        nc.vector.tensor_scalar(
                            t[:, sl], stg[:, sl], 1.0, None,
                            op0=ALU.mult, op1=ALU.add, accum_out=p[:])
                        ps.append(p)
                else:
                    t = xpad_pool.tile([128, HP * WP], F32, tag="xpad",
                                       name=f"xp{b}_{cc}")
                    nc.sync.dma_start(t[:], xc[:])
                    p = small_pool.tile([128, 1], F32, tag="pooled", bufs=8,
                                        name=f"pool{b}_{cc}")
                    nc.vector.reduce_sum(out=p[:], in_=t[:], axis=AX.XY)
                    ps.append(p)
                xv[(b, cc)] = t.rearrange("p (h w) -> p h w", w=WP)
                pooled_t[(b, cc)] = ps

        def emit_routing(b):
            mps = mpsum_pool.tile([128, 512], F32, tag="mps", name=f"mps{b}")
            parts = [(cc, p) for cc in range(CCH)
                     for p in pooled_t[(b, cc)]]
            for i, (cc, p) in enumerate(parts):
                nc.tensor.matmul(mps[0:HID, 0:1], rw1t_t[cc], p[:],
                                 start=(i == 0), stop=(i == len(parts) - 1))
            h_sb = small_pool.tile([HID, 1], F32, tag="h", name=f"h{b}")
            nc.scalar.activation(h_sb[:], mps[0:HID, 0:1], AF.Relu, bias=rb1_t[:])
            nc.tensor.matmul(mps[0:1, 4:4 + E], h_sb[:], rw2t_t[:],
                             start=True, stop=True)
            ze = small_pool.tile([1, E], F32, tag="ze", name=f"ze{b}")
            nc.vector.tensor_add(ze[:], mps[0:1, 4:4 + E], rb2_t[:])
            es = small_pool.tile([1, E], F32, tag="es", name=f"es{b}")
            nc.scalar.activation(es[:], ze[:], AF.Exp)
            # mix with UNNORMALIZED exp weights; 1/sum is applied later as
            # the PSUM-evacuation scale (keeps softmax off the critical path)
            nc.tensor.matmul(mps[0:128, 8:8 + E], ones_t[:], es[:],
                             start=True, stop=True)
            rbc = small_pool.tile([128, E], F32, tag="rbc", name=f"rbc{b}")
            nc.scalar.copy(rbc[:], mps[0:128, 8:8 + E])
            ssum = small_pool.tile([1, 1], F32, tag="ssum", name=f"ss{b}")
            nc.vector.reduce_sum(out=ssum[:], in_=es[:], axis=AX.X)
            rec = small_pool.tile([1, 1], F32, tag="rec", name=f"rec{b}")
            nc.vector.reciprocal(rec[:], ssum[:])
            nc.tensor.matmul(mps[0:128, 12:13], ones_t[:], rec[:],
                             start=True, stop=True)
            rinv = small_pool.tile([128, 1], F32, tag="rinv", name=f"ri{b}")
            nc.scalar.copy(rinv[:], mps[0:128, 12:13])
            return rbc, rinv

        def emit_mixing(b, rbc):
            seg = KK * O
            for cc in range(CCH):
                slab = slabs[cc]
                cmb = comb_pool.tile([128, seg], mmdt, tag="comb",
                                     name=f"cmb{b}_{cc}")
                for g in range(NG):
                    lo = g * GSZ
                    a = scr_pool.tile([128, GSZ], F32, tag="scr",
                                      name=f"scr{b}_{cc}_{g}")
                    nc.vector.tensor_scalar_mul(
                        a[:], slab[0][g][:], rbc[:, 0:1])
                    for e in range(1, E - 1):
                        nc.vector.scalar_tensor_tensor(
                            a[:], slab[e][g][:],
                            rbc[:, e:e + 1], a[:], op0=ALU.mult, op1=ALU.add)
                    nc.vector.scalar_tensor_tensor(
                        cmb[:, lo:lo + GSZ],
                        slab[E - 1][g][:],
                        rbc[:, E - 1:E], a[:], op0=ALU.mult, op1=ALU.add)
                comb[(b, cc)] = cmb

        def emit_conv_ochunk(b, oc, rinv):
            ptiles = [cpsum_pool.tile([128, NBLK], F32, tag="cps",
                                      name=f"cp{b}_{oc}_{rb}")
                      for rb in range(NRB)]
            for cc in range(CCH):
                cmb = comb[(b, cc)]
                xvc = xv[(b, cc)]
                for ij in range(KK):
                    di, dj = ij // K, ij % K
                    w_ap = cmb[:, ij * O + oc * 128: ij * O + oc * 128 + 128]
                    first = (cc == 0 and ij == 0)
                    last = (cc == CCH - 1 and ij == KK - 1)
                    for rb in range(NRB):
                        rhs = xvc[:, rb * RB_ROWS + di: rb * RB_ROWS + di + RB_ROWS,
                                  dj: dj + W]
                        nc.tensor.matmul(ptiles[rb][:], w_ap, rhs,
                                         start=first, stop=last)
            for rb in range(NRB):
                st = ostg_pool.tile([128, NBLK], F32, tag="ostg",
                                    name=f"st{b}_{oc}_{rb}")
                # evac applies the deferred softmax normalization; alternate
                # engines so the tail drains twice as fast
                if rb % 2 == 0:
                    nc.scalar.mul(st[:], ptiles[rb][:], rinv[:, 0:1])
                else:
                    nc.vector.tensor_scalar_mul(st[:], ptiles[rb][:],
                                                rinv[:, 0:1])
                nc.sync.dma_start(
                    out_d[b, oc * 128:(oc + 1) * 128,
                          rb * RB_ROWS:(rb + 1) * RB_ROWS, :],
                    st[:])

        # ---- emission: software-pipelined across samples ----
        def emit_pipeline():
            emit_loads(0)
            emit_slab_loads()
            route = {0: emit_routing(0)}
            emit_mixing(0, route[0][0])
            for b in range(BL):
                if b + 1 < BL:
                    emit_loads(b + 1)
                emit_conv_ochunk(b, 0, route[b][1])
                if b + 1 < BL:
                    route[b + 1] = emit_routing(b + 1)
                emit_conv_ochunk(b, 1, route[b][1])
                if b + 1 < BL:
                    emit_mixing(b + 1, route[b + 1][0])

        if loop_n is not None:
            # on-device HW loop around the whole pipeline (for timing)
            with tc.For_i(0, loop_n, 1):
                emit_pipeline()
        else:
            for _rep in range(reps):
                emit_pipeline()

    nc.compile()
    return nc


def _prep_inputs(x, experts, rw1, rb1, rw2, rb2):
    x = np.asarray(x, dtype=np.float32)
    x = np.ascontiguousarray(
        np.pad(x, ((0, 0), (0, 0), (1, 1), (1, 1))))
    experts = np.asarray(experts, dtype=np.float32)
    # [E,O,C,K,K] -> [C,E,K*K,O]
    ex_t = np.ascontiguousarray(
        np.transpose(experts, (2, 0, 3, 4, 1)).reshape(C, E, KK, O))
    rw1t = (np.asarray(rw1, dtype=np.float32) / float(H * W)).T  # [C, HID]
    rb1v = np.asarray(rb1, dtype=np.float32)
    rw2t = np.asarray(rw2, dtype=np.float32).T                   # [HID, E]
    rb2v = np.asarray(rb2, dtype=np.float32)
    RP = 2 * HID + 1 + 2 * E
    rp = np.zeros((128, RP), np.float32)
    rp[:, 0:HID] = rw1t[0:128]
    rp[:, HID:2 * HID] = rw1t[128:256]
    rp[0:HID, 2 * HID] = rb1v
    rp[0:HID, 2 * HID + 1:2 * HID + 1 + E] = rw2t
    rp[0, 2 * HID + 1 + E:2 * HID + 1 + 2 * E] = rb2v
    in_maps = []
    for i in range(NCORES):
        in_maps.append({
            "x": np.ascontiguousarray(x[i * BL:(i + 1) * BL]),
            "experts_t": ex_t,
            "rparams": rp,
        })
    return in_maps


def run(inputs, trace=False, use_f32r=True, **trace_kwargs):
    """Build (cached), run on 8 cores, return (full_out, BassKernelResults)."""
    key = ("prog", use_f32r)
    if key not in _CACHE:
        _CACHE[key] = _build_program(use_f32r=use_f32r)
    nc = _CACHE[key]
    in_maps = _prep_inputs(**inputs)
    res = run_bass_kernel_spmd(nc, in_maps, list(range(NCORES)),
                               trace=trace, **trace_kwargs)
    out = np.concatenate([res.results[i]["out"] for i in range(NCORES)], axis=0)
    return out, res


def kernel(x, experts, rw1, rb1, rw2, rb2):
    out, _ = run(dict(x=x, experts=experts, rw1=rw1, rb1=rb1, rw2=rw2, rb2=rb2))
    return out

